# revision 25
# baseline (speedup 1.0000x reference)
"""Exact sliding-window causal attention for Trainium2, sharded over 8 NeuronCores.

Problem: B=16, S=4096, D=64, window=512, causal. Each core handles 2 batches
(batch-parallel sharding, no cross-core communication).

Per-core algorithm:
  - q,k are loaded fp32, cast to bf16 on the vector engine, stored to a
    128-col padded DRAM scratch, and transposed into [d, s] layout with one
    xbar DMA-transpose each (the contraction dim must sit on partitions for
    the QK^T matmul). Loads run on the SP HWDGE ring, store+transpose on the
    ACT ring so the two chains overlap.
  - v is cast to bf16 with a constant ones column appended at index D, so the
    PV matmul also produces the softmax denominator.
  - Key tiles are processed in pairs: for each pair (kt, kt+1), four QK
    matmuls produce scores^T for the ten (query-tile, key-tile) combinations
    in one 3-bank fp32 PSUM tile, then a single 1280-wide exp on the scalar
    engine writes bf16 probs (scores ~ N(0,1), so no max-subtraction is
    needed). The two triangular masks (causal diagonal tiles, sliding-window
    far tiles) are applied post-exp by gpsimd.affine_select (0/1, exact).
  - Each query tile's 5 PV matmuls run as one consecutive fp32-PSUM
    accumulation group (probs stationary, [v|1] moving), software-pipelined
    two key-tiles behind QK so the in-order PE queue never waits on the
    scalar engine. A [128,1] reciprocal + tensor_scalar_mul normalizes into
    the output tile. All accumulation/normalization is fp32.
  - A ~11us burst of dummy matmuls at kernel start (under the DMA prologue)
    flips the PE HAM clock gate to full rate.
"""

import sys

if "/opt/trn_rl_repo" not in sys.path:
    sys.path.insert(0, "/opt/trn_rl_repo")

from contextlib import ExitStack

import numpy as np

import concourse.bass as bass
import concourse.tile as tile
from concourse import mybir
from concourse.bass_utils import run_bass_kernel_spmd

B, S, D = 16, 4096, 64
WINDOW = 512
N_CORES = 8
B_LOCAL = B // N_CORES  # batches per core
NT = S // 128  # 32 query/key tiles per batch
F32 = mybir.dt.float32
BF16 = mybir.dt.bfloat16


# --- workaround: this walrus build accepts at most ONE sync-wait command per
# instruction. After Tile has assigned semaphores, hoist excess waits onto
# same-engine NoOp instructions inserted immediately before the instruction
# (the engine blocks on each in program order — identical semantics).
def _split_multi_waits(nc, max_waits=1):
    n_split = 0
    for f in nc.m.functions:
        for bb in f.blocks:
            insts = bb.instructions
            i = 0
            while i < len(insts):
                inst = insts[i]
                si = inst.sync_info
                if si is not None and si.on_wait and len(si.on_wait) > max_waits:
                    waits = list(si.on_wait)
                    si.on_wait = waits[-max_waits:]
                    for jj, w in enumerate(waits[:-max_waits]):
                        nop = mybir.InstNoOp(
                            name=f"{inst.name}_wnop{jj}", ins=[], outs=[]
                        )
                        nop.engine = inst.engine
                        nop.sync_info = mybir.SyncInfo(on_wait=[w], on_update=[])
                        insts.insert(i, nop)
                        i += 1
                        n_split += 1
                i += 1
    return n_split


def build(n_warmup=18):
    nc = bass.Bass(
        "TRN2", target_bir_lowering=False, debug=False, num_devices=N_CORES
    )
    q = nc.dram_tensor("q", [B_LOCAL, S, D], F32, kind="ExternalInput").ap()
    k = nc.dram_tensor("k", [B_LOCAL, S, D], F32, kind="ExternalInput").ap()
    v = nc.dram_tensor("v", [B_LOCAL, S, D], F32, kind="ExternalInput").ap()
    out = nc.dram_tensor("out", [B_LOCAL, S, D], F32, kind="ExternalOutput").ap()
    scale = float(D) ** -0.5

    with tile.TileContext(nc) as tc, ExitStack() as ctx:
        singles = ctx.enter_context(tc.tile_pool(name="singles", bufs=1))
        tqp = ctx.enter_context(tc.tile_pool(name="tqp", bufs=2))
        probsp = ctx.enter_context(tc.tile_pool(name="probsp", bufs=8))
        outp = ctx.enter_context(tc.tile_pool(name="outp", bufs=2))
        vldp = ctx.enter_context(tc.tile_pool(name="vldp", bufs=3))
        recp = ctx.enter_context(tc.tile_pool(name="recp", bufs=4))
        dramp = ctx.enter_context(tc.tile_pool(name="dramp", bufs=2, space="DRAM"))
        psp = ctx.enter_context(tc.tile_pool(name="psp", bufs=3, space="PSUM"))
        accp = ctx.enter_context(tc.tile_pool(name="accp", bufs=2, space="PSUM"))

        # v with a ones column at index D: the PV matmul then yields the
        # softmax denominator in accumulator column D. memset once; loads only
        # overwrite the first D columns each batch.
        v_ext = []
        for i in range(min(2, B_LOCAL)):
            ve = singles.tile([128, NT, D + 2], BF16, tag=f"vext{i}")
            nc.vector.memset(ve, 1.0)
            v_ext.append(ve)

        # HAM warm-up: dummy matmuls at kernel start (overlapping the input
        # DMA prologue) flip the PE clock gate to 8/8 (2.4 GHz).
        wu = singles.tile([128, 512], BF16, tag="wu")
        nc.vector.memset(wu, 0.0)
        wu_ps = psp.tile([128, 640], F32, tag="ps", name="wu_ps")
        for i in range(n_warmup):
            off = 128 * (i % 2)
            nc.tensor.matmul(
                wu_ps[:, off : off + 512],
                lhsT=wu[:, 0:128],
                rhs=wu[:, 0:512],
                start=True,
                stop=True,
            )

        # stage all batches' inputs up front so batch b+1's DMAs overlap
        # batch b's compute.
        qTs, kTs, ves = [], [], []
        for b in range(B_LOCAL):
            ve = v_ext[b % 2]
            # interleave q,k as [q|k] and [k|q] 128-col DRAM scratches so
            # the bf16 stores are fully contiguous 256B rows, then one xbar
            # transpose each: qkT[0:64] = q^T (matmul rhs, base 0) and
            # kqT[0:64] = k^T (matmul lhsT, base 0).
            qkpad = dramp.tile([S, 128], BF16, tag="qkpad")
            kqpad = dramp.tile([S, 128], BF16, tag="kqpad")
            qkT = tqp.tile([128, S], BF16, tag="qkT")
            kqT = tqp.tile([128, S], BF16, tag="kqT")
            qk_bf = vldp.tile([128, NT, 2, D], BF16, tag="qkbf")
            kq_bf = vldp.tile([128, NT, 2, D], BF16, tag="kqbf")
            qkpv = qkpad.rearrange("(n p) c -> p n c", p=128)
            kqpv = kqpad.rearrange("(n p) c -> p n c", p=128)
            qv_ = q[b].rearrange("(n p) d -> p n d", p=128)
            kv_ = k[b].rearrange("(n p) d -> p n d", p=128)
            dge = nc.scalar if b == 0 else nc.sync
            nch = 4 if b == 0 else 1
            cn = NT // nch
            for ci in range(nch):
                cs = slice(ci * cn, (ci + 1) * cn)
                rs = slice(ci * cn * 128, (ci + 1) * cn * 128)
                q_sb = vldp.tile([128, cn, D], F32, tag="insb0", name=f"qsb_{b}_{ci}")
                k_sb = vldp.tile([128, cn, D], F32, tag="insb1", name=f"ksb_{b}_{ci}")
                nc.sync.dma_start(q_sb[:], qv_[:, cs, :])
                nc.vector.tensor_copy(qk_bf[:, cs, 0, :], q_sb[:])
                nc.vector.tensor_copy(kq_bf[:, cs, 1, :], q_sb[:])
                nc.sync.dma_start(k_sb[:], kv_[:, cs, :])
                nc.vector.tensor_copy(qk_bf[:, cs, 1, :], k_sb[:])
                nc.vector.tensor_copy(kq_bf[:, cs, 0, :], k_sb[:])
                dge.dma_start(qkpv[:, cs, :], qk_bf[:, cs, :, :])
                dge.dma_start_transpose(qkT[:, rs], qkpad[rs, :])
                dge.dma_start(kqpv[:, cs, :], kq_bf[:, cs, :, :])
                dge.dma_start_transpose(kqT[:, rs], kqpad[rs, :])
            # v: fp32 load + DVE cast into the ones-extended tile
            v_sb = vldp.tile([128, NT, D], F32, tag="vsb")
            nc.sync.dma_start(v_sb[:], v[b].rearrange("(n p) d -> p n d", p=128))
            nc.vector.tensor_copy(ve[:, :, 0:D], v_sb[:])
            qTs.append(qkT)
            kTs.append(kqT)
            ves.append(ve)

        for b in range(B_LOCAL):
            ve = ves[b]
            qT = qTs[b]
            kT = kTs[b]
            out_sb = outp.tile([128, NT * D], F32)
            # probs_hist: kt -> (tile, col base of slots 0-3, col base of slot 4)
            probs_hist = {}

            def emit_pv(qt, b=b, ve=ve, out_sb=out_sb, probs_hist=probs_hist):
                lo = max(0, qt - 4)
                acc = accp.tile(
                    [128, D + 2], F32, tag="acc", name=f"acc_{b}_{qt}"
                )
                for kt2 in range(lo, qt + 1):
                    pt, b03, b4 = probs_hist[kt2]
                    s = qt - kt2  # slot of qt within probs[kt2]
                    col = b03 + s * 128 if s < 4 else b4
                    nc.tensor.matmul(
                        acc[:],
                        lhsT=pt[:, col : col + 128],
                        rhs=ve[:, kt2, :],
                        start=(kt2 == lo),
                        stop=(kt2 == qt),
                    )
                probs_hist.pop(qt - 4, None)
                rec = recp.tile([128, 1], F32, tag="rec", name=f"rec_{b}_{qt}")
                nc.vector.reciprocal(rec, acc[:, D : D + 1])
                nc.vector.tensor_scalar_mul(
                    out_sb[:, qt * D : (qt + 1) * D],
                    acc[:, 0:D],
                    rec,
                )

            def diag_mask(ap):
                # scores^T layout: partition=key j (within tile), free=query i;
                # causal diagonal tile keeps j <= i  <=>  (i - j) >= 0
                nc.gpsimd.affine_select(
                    out=ap,
                    in_=ap,
                    compare_op=mybir.AluOpType.is_ge,
                    fill=0.0,
                    base=0,
                    pattern=[[1, 128]],
                    channel_multiplier=-1,
                )

            def far_mask(ap, ntiles):
                # window-start tile(s): keep j >= i+1  <=>  (j - i - 1) >= 0
                nc.gpsimd.affine_select(
                    out=ap,
                    in_=ap,
                    compare_op=mybir.AluOpType.is_ge,
                    fill=0.0,
                    base=-1,
                    pattern=[[0, ntiles], [-1, 128]],
                    channel_multiplier=1,
                )

            for kt in range(NT):
                nslots = min(NT - kt, 5)
                w4 = min(4, nslots) * 128
                scores = psp.tile([128, 640], F32, tag="ps")
                kT_t = kT[0:64, kt * 128 : (kt + 1) * 128]
                nc.tensor.matmul(
                    scores[:, 0:w4],
                    lhsT=kT_t,
                    rhs=qT[0:64, kt * 128 : kt * 128 + w4],
                    start=True,
                    stop=True,
                )
                if nslots == 5:
                    nc.tensor.matmul(
                        scores[:, 512:640],
                        lhsT=kT_t,
                        rhs=qT[0:64, (kt + 4) * 128 : (kt + 5) * 128],
                        start=True,
                        stop=True,
                    )
                probs = probsp.tile([128, 640], BF16)
                nc.scalar.activation(
                    probs[:, 0 : nslots * 128],
                    scores[:, 0 : nslots * 128],
                    mybir.ActivationFunctionType.Exp,
                    scale=scale,
                )
                diag_mask(probs[:, 0:128])
                if nslots == 5:
                    far_mask(probs[:, 512:640], 1)
                probs_hist[kt] = (probs, 0, 512)
                # PV two key-tiles behind so the in-order PE queue never
                # waits on ACT/gpsimd for this iteration's probs
                if kt >= 2:
                    emit_pv(kt - 2)
            emit_pv(NT - 2)
            emit_pv(NT - 1)
            nc.sync.dma_start(
                out[b].rearrange("(n p) d -> p n d", p=128),
                out_sb.rearrange("p (n d) -> p n d", d=D),
            )
    _split_multi_waits(nc)
    return nc


_CACHE = {}


def _get_nc():
    if "nc" not in _CACHE:
        _CACHE["nc"] = build()
    return _CACHE["nc"]


def _make_in_maps(q, k, v):
    q = np.ascontiguousarray(np.asarray(q, dtype=np.float32))
    k = np.ascontiguousarray(np.asarray(k, dtype=np.float32))
    v = np.ascontiguousarray(np.asarray(v, dtype=np.float32))
    return [
        {
            "q": np.ascontiguousarray(q[c * B_LOCAL : (c + 1) * B_LOCAL]),
            "k": np.ascontiguousarray(k[c * B_LOCAL : (c + 1) * B_LOCAL]),
            "v": np.ascontiguousarray(v[c * B_LOCAL : (c + 1) * B_LOCAL]),
        }
        for c in range(N_CORES)
    ]


def kernel(q, k, v):
    nc = _get_nc()
    res = run_bass_kernel_spmd(nc, _make_in_maps(q, k, v), core_ids=list(range(N_CORES)))
    return np.concatenate(
        [res.results[c]["out"] for c in range(N_CORES)], axis=0
    )


# revision 26
# speedup vs baseline: 1.2750x; 1.2750x over previous
"""Exact sliding-window causal attention for Trainium2, sharded over 8 NeuronCores.

Problem: B=16, S=4096, D=64, window=512, causal. Each core handles 2 batches
(batch-parallel sharding, no cross-core communication).

Per-core algorithm:
  - q,k are loaded fp32, cast to bf16 on the vector engine, stored to a
    128-col padded DRAM scratch, and transposed into [d, s] layout with one
    xbar DMA-transpose each (the contraction dim must sit on partitions for
    the QK^T matmul). Loads run on the SP HWDGE ring, store+transpose on the
    ACT ring so the two chains overlap.
  - v is cast to bf16 with a constant ones column appended at index D, so the
    PV matmul also produces the softmax denominator.
  - Key tiles are processed in pairs: for each pair (kt, kt+1), four QK
    matmuls produce scores^T for the ten (query-tile, key-tile) combinations
    in one 3-bank fp32 PSUM tile, then a single 1280-wide exp on the scalar
    engine writes bf16 probs (scores ~ N(0,1), so no max-subtraction is
    needed). The two triangular masks (causal diagonal tiles, sliding-window
    far tiles) are applied post-exp by gpsimd.affine_select (0/1, exact).
  - Each query tile's 5 PV matmuls run as one consecutive fp32-PSUM
    accumulation group (probs stationary, [v|1] moving), software-pipelined
    two key-tiles behind QK so the in-order PE queue never waits on the
    scalar engine. A [128,1] reciprocal + tensor_scalar_mul normalizes into
    the output tile. All accumulation/normalization is fp32.
  - A ~11us burst of dummy matmuls at kernel start (under the DMA prologue)
    flips the PE HAM clock gate to full rate.
"""

import sys

if "/opt/trn_rl_repo" not in sys.path:
    sys.path.insert(0, "/opt/trn_rl_repo")

from contextlib import ExitStack

import numpy as np

import concourse.bass as bass
import concourse.tile as tile
from concourse import mybir
from concourse.bass_utils import run_bass_kernel_spmd

B, S, D = 16, 4096, 64
WINDOW = 512
N_CORES = 8
B_LOCAL = B // N_CORES  # batches per core
NT = S // 128  # 32 query/key tiles per batch
F32 = mybir.dt.float32
BF16 = mybir.dt.bfloat16


# --- workaround: this walrus build accepts at most ONE sync-wait command per
# instruction. After Tile has assigned semaphores, hoist excess waits onto
# same-engine NoOp instructions inserted immediately before the instruction
# (the engine blocks on each in program order — identical semantics).
def _split_multi_waits(nc, max_waits=1):
    n_split = 0
    for f in nc.m.functions:
        for bb in f.blocks:
            insts = bb.instructions
            i = 0
            while i < len(insts):
                inst = insts[i]
                si = inst.sync_info
                if si is not None and si.on_wait and len(si.on_wait) > max_waits:
                    waits = list(si.on_wait)
                    si.on_wait = waits[-max_waits:]
                    for jj, w in enumerate(waits[:-max_waits]):
                        nop = mybir.InstNoOp(
                            name=f"{inst.name}_wnop{jj}", ins=[], outs=[]
                        )
                        nop.engine = inst.engine
                        nop.sync_info = mybir.SyncInfo(on_wait=[w], on_update=[])
                        insts.insert(i, nop)
                        i += 1
                        n_split += 1
                i += 1
    return n_split


def build():
    nc = bass.Bass(
        "TRN2", target_bir_lowering=False, debug=False, num_devices=N_CORES
    )
    q = nc.dram_tensor("q", [B_LOCAL, S, D], F32, kind="ExternalInput").ap()
    k = nc.dram_tensor("k", [B_LOCAL, S, D], F32, kind="ExternalInput").ap()
    v = nc.dram_tensor("v", [B_LOCAL, S, D], F32, kind="ExternalInput").ap()
    out = nc.dram_tensor("out", [B_LOCAL, S, D], F32, kind="ExternalOutput").ap()
    scale = float(D) ** -0.5

    with tile.TileContext(nc) as tc, ExitStack() as ctx:
        singles = ctx.enter_context(tc.tile_pool(name="singles", bufs=1))
        tqp = ctx.enter_context(tc.tile_pool(name="tqp", bufs=2))
        probsp = ctx.enter_context(tc.tile_pool(name="probsp", bufs=8))
        outp = ctx.enter_context(tc.tile_pool(name="outp", bufs=2))
        vldp = ctx.enter_context(tc.tile_pool(name="vldp", bufs=3))
        recp = ctx.enter_context(tc.tile_pool(name="recp", bufs=4))
        dramp = ctx.enter_context(tc.tile_pool(name="dramp", bufs=2, space="DRAM"))
        psp = ctx.enter_context(tc.tile_pool(name="psp", bufs=3, space="PSUM"))
        accp = ctx.enter_context(tc.tile_pool(name="accp", bufs=2, space="PSUM"))

        # v with a ones column at index D: the PV matmul then yields the
        # softmax denominator in accumulator column D. memset once; loads only
        # overwrite the first D columns each batch.
        v_ext = []
        for i in range(min(2, B_LOCAL)):
            ve = singles.tile([128, NT, D + 2], BF16, tag=f"vext{i}")
            nc.vector.memset(ve, 1.0)
            v_ext.append(ve)

        ident = singles.tile([128, 128], F32)
        nc.vector.memset(ident, 1.0)
        nc.gpsimd.affine_select(
            out=ident,
            in_=ident,
            compare_op=mybir.AluOpType.is_equal,
            fill=0.0,
            base=0,
            pattern=[[-1, 128]],
            channel_multiplier=1,
        )

        # stage inputs. Batch 0 is latency-critical: PE transposes (they both
        # warm the HAM clock gate and start producing qT/kT within ~8us).
        # Batch 1 has ~60us of slack: zero-PE-cost xbar DMA transposes.
        qTs, kTs, ves = [], [], []

        # --- batch 0: chunked fp32 loads -> PE transpose -> DVE cast-evac
        b = 0
        ve0 = v_ext[0]
        qT0 = tqp.tile([128, S], BF16, tag="qkT", name="qT_b0")
        kT0 = tqp.tile([128, S], BF16, tag="kqT", name="kT_b0")
        qv_ = q[0].rearrange("(n p) d -> p n d", p=128)
        kv_ = k[0].rearrange("(n p) d -> p n d", p=128)
        nch = 4
        cn = NT // nch
        for ci in range(nch):
            cs = slice(ci * cn, (ci + 1) * cn)
            for srcv, tmat, tg in ((qv_, qT0, 0), (kv_, kT0, 1)):
                t_sb = vldp.tile(
                    [128, cn, D], F32, tag=f"insb{tg}", name=f"insb_{tg}_{ci}"
                )
                nc.sync.dma_start(t_sb[:], srcv[:, cs, :])
                for g in range(cn // 4):
                    tp = psp.tile([128, 640], F32, tag="ps", name=f"tp_{tg}_{ci}_{g}")
                    for j in range(4):
                        nc.tensor.transpose(
                            tp[0:64, j * 128 : (j + 1) * 128],
                            t_sb[:, g * 4 + j, :],
                            ident,
                        )
                    col = (ci * cn + g * 4) * 128
                    nc.vector.tensor_copy(
                        tmat[0:64, col : col + 512], tp[0:64, 0:512]
                    )
        v_sb = vldp.tile([128, NT, D], F32, tag="vsb", name="vsb_0")
        nc.sync.dma_start(v_sb[:], v[0].rearrange("(n p) d -> p n d", p=128))
        nc.vector.tensor_copy(ve0[:, :, 0:D], v_sb[:])
        qTs.append(qT0)
        kTs.append(kT0)
        ves.append(ve0)

        # --- batch 1: DRAM bf16 [q|k]/[k|q] scratches + xbar transposes
        if B_LOCAL > 1:
            b = 1
            ve1 = v_ext[1]
            qkpad = dramp.tile([S, 128], BF16, tag="qkpad")
            kqpad = dramp.tile([S, 128], BF16, tag="kqpad")
            qkT = tqp.tile([128, S], BF16, tag="qkT", name="qT_b1")
            kqT = tqp.tile([128, S], BF16, tag="kqT", name="kT_b1")
            qk_bf = vldp.tile([128, NT, 2, D], BF16, tag="qkbf")
            kq_bf = vldp.tile([128, NT, 2, D], BF16, tag="kqbf")
            for src_t, half in ((q, 0), (k, 1)):
                t_sb = vldp.tile(
                    [128, NT, D], F32, tag=f"in1sb{half}", name=f"in1sb_{half}"
                )
                nc.sync.dma_start(
                    t_sb[:], src_t[1].rearrange("(n p) d -> p n d", p=128)
                )
                nc.vector.tensor_copy(qk_bf[:, :, half, :], t_sb[:])
                nc.vector.tensor_copy(kq_bf[:, :, 1 - half, :], t_sb[:])
            nc.sync.dma_start(
                qkpad.rearrange("(n p) c -> p n c", p=128), qk_bf[:]
            )
            nc.sync.dma_start_transpose(qkT[:], qkpad[:])
            nc.sync.dma_start(
                kqpad.rearrange("(n p) c -> p n c", p=128), kq_bf[:]
            )
            nc.sync.dma_start_transpose(kqT[:], kqpad[:])
            v_sb1 = vldp.tile([128, NT, D], F32, tag="vsb", name="vsb_1")
            nc.sync.dma_start(v_sb1[:], v[1].rearrange("(n p) d -> p n d", p=128))
            nc.vector.tensor_copy(ve1[:, :, 0:D], v_sb1[:])
            qTs.append(qkT)
            kTs.append(kqT)
            ves.append(ve1)

        for b in range(B_LOCAL):
            ve = ves[b]
            qT = qTs[b]
            kT = kTs[b]
            out_sb = outp.tile([128, NT * D], F32)
            # probs_hist: kt -> (tile, col base of slots 0-3, col base of slot 4)
            probs_hist = {}

            def emit_pv(qt, b=b, ve=ve, out_sb=out_sb, probs_hist=probs_hist):
                lo = max(0, qt - 4)
                acc = accp.tile(
                    [128, D + 2], F32, tag="acc", name=f"acc_{b}_{qt}"
                )
                for kt2 in range(lo, qt + 1):
                    pt, b03, b4 = probs_hist[kt2]
                    s = qt - kt2  # slot of qt within probs[kt2]
                    col = b03 + s * 128 if s < 4 else b4
                    nc.tensor.matmul(
                        acc[:],
                        lhsT=pt[:, col : col + 128],
                        rhs=ve[:, kt2, :],
                        start=(kt2 == lo),
                        stop=(kt2 == qt),
                    )
                probs_hist.pop(qt - 4, None)
                rec = recp.tile([128, 1], F32, tag="rec", name=f"rec_{b}_{qt}")
                nc.vector.reciprocal(rec, acc[:, D : D + 1])
                nc.vector.tensor_scalar_mul(
                    out_sb[:, qt * D : (qt + 1) * D],
                    acc[:, 0:D],
                    rec,
                )

            def diag_mask(ap):
                # scores^T layout: partition=key j (within tile), free=query i;
                # causal diagonal tile keeps j <= i  <=>  (i - j) >= 0
                nc.gpsimd.affine_select(
                    out=ap,
                    in_=ap,
                    compare_op=mybir.AluOpType.is_ge,
                    fill=0.0,
                    base=0,
                    pattern=[[1, 128]],
                    channel_multiplier=-1,
                )

            def far_mask(ap, ntiles):
                # window-start tile(s): keep j >= i+1  <=>  (j - i - 1) >= 0
                nc.gpsimd.affine_select(
                    out=ap,
                    in_=ap,
                    compare_op=mybir.AluOpType.is_ge,
                    fill=0.0,
                    base=-1,
                    pattern=[[0, ntiles], [-1, 128]],
                    channel_multiplier=1,
                )

            for kt in range(NT):
                nslots = min(NT - kt, 5)
                w4 = min(4, nslots) * 128
                scores = psp.tile([128, 640], F32, tag="ps")
                kT_t = kT[0:64, kt * 128 : (kt + 1) * 128]
                nc.tensor.matmul(
                    scores[:, 0:w4],
                    lhsT=kT_t,
                    rhs=qT[0:64, kt * 128 : kt * 128 + w4],
                    start=True,
                    stop=True,
                )
                if nslots == 5:
                    nc.tensor.matmul(
                        scores[:, 512:640],
                        lhsT=kT_t,
                        rhs=qT[0:64, (kt + 4) * 128 : (kt + 5) * 128],
                        start=True,
                        stop=True,
                    )
                probs = probsp.tile([128, 640], BF16)
                nc.scalar.activation(
                    probs[:, 0 : nslots * 128],
                    scores[:, 0 : nslots * 128],
                    mybir.ActivationFunctionType.Exp,
                    scale=scale,
                )
                diag_mask(probs[:, 0:128])
                if nslots == 5:
                    far_mask(probs[:, 512:640], 1)
                probs_hist[kt] = (probs, 0, 512)
                # PV two key-tiles behind so the in-order PE queue never
                # waits on ACT/gpsimd for this iteration's probs
                if kt >= 2:
                    emit_pv(kt - 2)
            emit_pv(NT - 2)
            emit_pv(NT - 1)
            nc.sync.dma_start(
                out[b].rearrange("(n p) d -> p n d", p=128),
                out_sb.rearrange("p (n d) -> p n d", d=D),
            )
    _split_multi_waits(nc)
    return nc


_CACHE = {}


def _get_nc():
    if "nc" not in _CACHE:
        _CACHE["nc"] = build()
    return _CACHE["nc"]


def _make_in_maps(q, k, v):
    q = np.ascontiguousarray(np.asarray(q, dtype=np.float32))
    k = np.ascontiguousarray(np.asarray(k, dtype=np.float32))
    v = np.ascontiguousarray(np.asarray(v, dtype=np.float32))
    return [
        {
            "q": np.ascontiguousarray(q[c * B_LOCAL : (c + 1) * B_LOCAL]),
            "k": np.ascontiguousarray(k[c * B_LOCAL : (c + 1) * B_LOCAL]),
            "v": np.ascontiguousarray(v[c * B_LOCAL : (c + 1) * B_LOCAL]),
        }
        for c in range(N_CORES)
    ]


def kernel(q, k, v):
    nc = _get_nc()
    res = run_bass_kernel_spmd(nc, _make_in_maps(q, k, v), core_ids=list(range(N_CORES)))
    return np.concatenate(
        [res.results[c]["out"] for c in range(N_CORES)], axis=0
    )
